# revision 44
# baseline (speedup 1.0000x reference)
"""Trainium2 Bass kernel for nn_Attention_50173807952647.

GQA attention block: qkv projections + partial interleaved RoPE + softmax
attention + output projection, fp32 inputs/outputs.

Sharding: 8 cores; core d owns kv-head d and query heads {2d, 2d+1} for all
4 batches (head/tensor parallel per the GQA grouping). Each core computes a
partial output (its heads' contribution through Wo); host sums partials + bias.

v2 design (bf16 matmul datapath, host-side transpose):
  x^T is transposed + cast to bf16 on the HOST and DMA'd directly into SBUF
  (eliminates all on-device x transposes and PSUM->SBUF copies).
  q^T = Wq_d.T x  [hd,t] bf16; kv^T likewise; RoPE in transposed layout with
  sign-folded bf16 sin/cos tables (stream_shuffle pair-swap on DVE).
  kT holds k^T duplicated on both partition halves so the score matmuls for
  the 2 query heads row-pack into the PE array (tile_position (0,0)/(64,0),
  concurrent, K=64 each) -> both heads' scores in one 512-cycle pass.
  P = exp(S^T * scale) on ACT straight PSUM->SBUF bf16 (no max subtraction:
  scores are ~N(0,1) after the 1/8 scale).
  O^T, denom = [v|1].T @ P accumulated in PSUM over k tiles (ones row in the
  v stationary gives denominators for free).
  normalize via reciprocal_approx + gpsimd partition_broadcast fused into the
  PSUM->SBUF oT assembly; out_partial[t,e] = oT.T @ Wo_d per q-block.
  Non-PE copies ride on the gpsimd/Pool engine to keep ACT exp-only.
"""

import sys

import numpy as np

HEADS = 16
KV_HEADS = 8
DIM_HEAD = 64
ROT_DIM = 32
SCALE = DIM_HEAD ** -0.5
B, N, DIM = 4, 2048, 1024
N_CORES = 8
T = B * N  # 8192 tokens
CHUNK = 512  # projection chunk (tokens)
QB = 512  # attention query block

_BUILT = {}


def _ensure_path():
    for p in ("/opt/trn_rl_repo",):
        if p not in sys.path:
            sys.path.insert(0, p)


def _rope_tables():
    """cos/sin tables [128, N] for the transposed [hd, t] layout.

    Row r (hd index within a core's 128 q-rows): head-local d = r % 64.
    d < ROT_DIM: cos(t * inv_freq[d//2]); sin with rotate-half sign folded
    (-sin on even d, +sin on odd d). Elsewhere cos=1, sin=0 so a single
    full-width mul+add applies RoPE only where it belongs.
    """
    inv_freq = 1.0 / (10000.0 ** (np.arange(0, ROT_DIM, 2, dtype=np.float64) / ROT_DIM))
    t = np.arange(N, dtype=np.float64)
    freqs = t[None, :] * inv_freq[:, None]  # [16, N]
    cos = np.ones((128, N), dtype=np.float64)
    sin = np.zeros((128, N), dtype=np.float64)
    for r in range(128):
        d = r % 64
        if d < ROT_DIM:
            f = freqs[d // 2]
            cos[r] = np.cos(f)
            sin[r] = (-1.0 if d % 2 == 0 else 1.0) * np.sin(f)
    return cos, sin


def _build(debug=False):
    if ("nc", debug) in _BUILT:
        return _BUILT[("nc", debug)]
    _ensure_path()
    import concourse.bass as bass  # noqa: F401
    import concourse.mybir as mybir
    import concourse.tile as tile
    from concourse import bacc
    from concourse.masks import make_identity

    dt = mybir.dt
    f32, bf16 = dt.float32, dt.bfloat16
    AF = mybir.ActivationFunctionType
    OP = mybir.AluOpType

    nc = bacc.Bacc("TRN2", target_bir_lowering=False, debug=False)

    xt_in = nc.dram_tensor("xt", [DIM, T], bf16, kind="ExternalInput").ap()
    wq_in = nc.dram_tensor("wq", [128, DIM], bf16, kind="ExternalInput").ap()
    wkv_in = nc.dram_tensor("wkv", [128, DIM], bf16, kind="ExternalInput").ap()
    wo_in = nc.dram_tensor("wo", [128, DIM], bf16, kind="ExternalInput").ap()
    cos_in = nc.dram_tensor("cos_t", [128, N], bf16, kind="ExternalInput").ap()
    sin_in = nc.dram_tensor("sin_t", [128, N], bf16, kind="ExternalInput").ap()
    out_d = nc.dram_tensor("out", [T, DIM], f32, kind="ExternalOutput").ap()
    if debug:
        dbg_qT = nc.dram_tensor("dbg_qT", [128, N], bf16, kind="ExternalOutput").ap()
        dbg_kT = nc.dram_tensor("dbg_kT", [128, N], bf16, kind="ExternalOutput").ap()
        dbg_v = nc.dram_tensor("dbg_v", [128, (N // 128) * 65], bf16, kind="ExternalOutput").ap()
        dbg_e = nc.dram_tensor("dbg_e", [128, 1024], bf16, kind="ExternalOutput").ap()
        dbg_oT = nc.dram_tensor("dbg_oT", [128, QB], bf16, kind="ExternalOutput").ap()

    NCH = N // CHUNK  # chunks per batch
    NQB = N // QB  # q blocks per batch
    NKT = N // 128  # key tiles per batch
    pair_mask = []
    for i in range(16):
        pair_mask += [2 * i + 1, 2 * i]

    with tile.TileContext(nc) as tc:
        with (
            tc.tile_pool(name="const", bufs=1) as constp,
            tc.tile_pool(name="xt", bufs=2) as xtp,
            tc.tile_pool(name="perbatch", bufs=2) as batchp,
            tc.tile_pool(name="rope", bufs=6) as ropep,
            tc.tile_pool(name="sm", bufs=5) as smp,
            tc.tile_pool(name="exp", bufs=5) as expp,
            tc.tile_pool(name="oup", bufs=2) as oup,
            tc.tile_pool(name="osb", bufs=4) as osbp,
            tc.tile_pool(name="outsb", bufs=2) as outsbp,
            tc.tile_pool(name="psS", bufs=2, space="PSUM") as psS,
            tc.tile_pool(name="psO", bufs=1, space="PSUM") as psO,
            tc.tile_pool(name="psP", bufs=2, space="PSUM") as psP,
        ):
            ident = constp.tile([64, 64], bf16)
            make_identity(nc, ident[:])
            wq_sb = constp.tile([128, DIM], bf16, tag="wq")
            wkv_sb = constp.tile([128, DIM], bf16, tag="wkv")
            wo_sb = constp.tile([128, DIM], bf16, tag="wo")
            cos_sb = constp.tile([128, N], bf16, tag="cos")
            sin_sb = constp.tile([128, N], bf16, tag="sin")

            def load_xt(b, xT, chunks=range(NCH), use_act=False):
                # column-chunked so chunk 0's operands land first; the lead-in
                # alternates the two HWDGE issue queues (SP / ACT) — in-loop
                # loads stay off ACT, which carries the exp stream
                for c in chunks:
                    for et in range(8):
                        eng = nc.scalar if (use_act and et % 2 == 1) else nc.sync
                        eng.dma_start(
                            xT[:, et * N + c * CHUNK: et * N + (c + 1) * CHUNK],
                            xt_in[et * 128:(et + 1) * 128,
                                  b * N + c * CHUNK: b * N + (c + 1) * CHUNK])

            # lead-in order: first the weights + chunk 0 of x^T (unblocks the
            # first projection ASAP), then the rest
            nc.sync.dma_start(wq_sb[:], wq_in[:])
            nc.scalar.dma_start(wkv_sb[:], wkv_in[:])
            xT0 = xtp.tile([128, 8 * N], bf16, tag="xT")
            load_xt(0, xT0, chunks=(0,), use_act=True)
            nc.sync.dma_start(cos_sb[:], cos_in[:])
            nc.scalar.dma_start(sin_sb[:], sin_in[:])
            nc.sync.dma_start(wo_sb[:], wo_in[:])
            load_xt(0, xT0, chunks=(1, 2, 3), use_act=True)

            def proj_gen(b, c, tiles, xT):
                """Generator: projections + rope for chunk c of batch b.

                Yields between PE ops so attn_core can interleave it into the
                PE-idle windows of the ACT-bound score/exp stream.
                """
                qT, kT, v_sb = tiles
                cs = slice(c * CHUNK, (c + 1) * CHUNK)
                qps = psP.tile([128, 512], f32, tag="psp")
                for et in range(8):
                    nc.tensor.matmul(qps[:],
                                     wq_sb[:, et * 128:(et + 1) * 128],
                                     xT[:, et * N + c * CHUNK:
                                        et * N + (c + 1) * CHUNK],
                                     start=(et == 0), stop=(et == 7))
                    if et % 2 == 1 and et < 7:
                        yield
                # rope epilogue: q (DVE) — emitted with the last q matmul so
                # the psP buffer's readers exist before its next reallocation
                shq = ropep.tile([128, CHUNK], f32, tag="ropef")
                nc.vector.stream_shuffle(shq[:], qps[:], pair_mask)
                t1q = ropep.tile([128, CHUNK], bf16, tag="rope")
                nc.vector.tensor_tensor(t1q[:], qps[:], cos_sb[:, cs], op=OP.mult)
                t2q = ropep.tile([128, CHUNK], bf16, tag="rope")
                nc.vector.tensor_tensor(t2q[:], shq[:], sin_sb[:, cs], op=OP.mult)
                nc.vector.tensor_tensor(qT[:, cs], t1q[:], t2q[:], op=OP.add)
                yield
                kvps = psP.tile([128, 512], f32, tag="psp")
                for et in range(8):
                    nc.tensor.matmul(kvps[:],
                                     wkv_sb[:, et * 128:(et + 1) * 128],
                                     xT[:, et * N + c * CHUNK:
                                        et * N + (c + 1) * CHUNK],
                                     start=(et == 0), stop=(et == 7))
                    if et % 2 == 1 and et < 7:
                        yield
                # rope epilogue: k -> kT rows 0:64, then duplicate to 64:128
                shk = ropep.tile([32, CHUNK], f32, tag="ropef")
                nc.vector.stream_shuffle(shk[:], kvps[0:32, :], pair_mask)
                t1k = ropep.tile([64, CHUNK], bf16, tag="rope")
                nc.vector.tensor_tensor(t1k[:], kvps[0:64, :], cos_sb[0:64, cs],
                                        op=OP.mult)
                t2k = ropep.tile([32, CHUNK], bf16, tag="rope")
                nc.vector.tensor_tensor(t2k[:], shk[:], sin_sb[0:32, cs], op=OP.mult)
                nc.vector.tensor_tensor(kT[0:32, cs], t1k[0:32, :], t2k[:], op=OP.add)
                nc.vector.tensor_copy(kT[32:64, cs], t1k[32:64, :])
                nc.sync.dma_start(kT[64:128, cs], kT[0:64, cs])
                vts = ropep.tile([64, CHUNK], bf16, tag="rope")
                nc.vector.tensor_copy(vts[:], kvps[64:128, :])
                yield
                # v fixup: transpose v^T [64, t] -> [t, 64] into v_sb (bf16)
                vtp = psP.tile([128, 512], f32, tag="psp")
                vtb = vtp[:].bitcast(bf16)  # [128, 1024] bf16 view
                for st in range(4):
                    nc.tensor.transpose(vtb[:, st * 128: st * 128 + 64],
                                        vts[:, st * 128:(st + 1) * 128],
                                        ident[:])
                    kt_i = c * 4 + st
                    nc.vector.tensor_copy(v_sb[:, kt_i * 65: kt_i * 65 + 64],
                                          vtb[:, st * 128: st * 128 + 64])
                    if st % 2 == 1:
                        yield

            def tail_gen(tiles, ops_t, es, out_cell):
                """Previous q-block's last two attnV pairs + the psO-draining
                copies, emitted INSIDE the next q-block's stream so they never
                sit between its first score matmuls in the PE queue."""
                qT, kT, v_sb = tiles
                for j in (NKT - 2, NKT - 1):
                    nc.tensor.matmul(ops_t[:, 0:512],
                                     v_sb[:, j * 65: j * 65 + 65],
                                     es[j][:, 0:512], start=False,
                                     stop=(j == NKT - 1))
                    nc.tensor.matmul(ops_t[:, 512:1024],
                                     v_sb[:, j * 65: j * 65 + 65],
                                     es[j][:, 512:1024], start=False,
                                     stop=(j == NKT - 1))
                    yield
                out_cell.append(attn_out_a(ops_t))
                yield

            def attn_core(b, qb, tiles, filler=None, outg=None, pend=None):
                qT, kT, v_sb = tiles
                qs = slice(qb * QB, (qb + 1) * QB)
                ops_t = psO.tile([65, 1024], f32, tag="ps_o")
                es = []
                for kt in range(NKT):
                    sps = psS.tile([128, 1024], f32, tag="ps_s")
                    # row-packed pair: head0 on array rows 0:63, head1 on
                    # 64:127 (kT duplicated halves) -> concurrent
                    nc.tensor.matmul(sps[:, 0:512],
                                     kT[0:64, kt * 128:(kt + 1) * 128],
                                     qT[0:64, qs], start=True, stop=True)
                    nc.tensor.matmul(sps[:, 512:1024],
                                     kT[64:128, kt * 128:(kt + 1) * 128],
                                     qT[64:128, qs], start=True, stop=True)
                    e_sb = expp.tile([128, 1024], bf16, tag="e")
                    nc.scalar.activation(e_sb[:], sps[:], AF.Exp, scale=SCALE)
                    if debug and b == 0 and qb == 0 and kt == 0:
                        nc.sync.dma_start(dbg_e[:], e_sb[:])
                    es.append(e_sb)
                    # previous q-block's tail (2 attnV pairs + psO drain)
                    if pend is not None and kt in (0, 1, 2):
                        next(pend, None)
                    if filler is not None:
                        next(filler, None)
                    # software-pipeline: attnV lags scores by 2 k-tiles so the
                    # exp dependency is already complete (no PE sem-wait stall)
                    if kt >= 2:
                        j = kt - 2
                        nc.tensor.matmul(ops_t[:, 0:512],
                                         v_sb[:, j * 65: j * 65 + 65],
                                         es[j][:, 0:512],
                                         start=(j == 0), stop=False)
                        nc.tensor.matmul(ops_t[:, 512:1024],
                                         v_sb[:, j * 65: j * 65 + 65],
                                         es[j][:, 512:1024],
                                         start=(j == 0), stop=False)
                    # out-projection of the q-block before that rides the
                    # PE-idle slack mid-stream, after the proj filler's psP
                    # readers are all emitted
                    if outg is not None:
                        if filler is not None:
                            if kt >= 10:
                                next(outg, None)
                                next(outg, None)
                        elif kt >= 8:
                            next(outg, None)
                return ops_t, es

            def attn_out_a(ops_t):
                """Cheap psO-draining part, emitted right after attn_core so
                the single psO buffer frees fast. Custom-DVE ops ignore a
                non-zero read partition base and DVE writes cannot shift
                partition base: stage via tensor_copy + SBUF DMA."""
                ou = oup.tile([64, 1024], bf16, tag="ou")
                nc.vector.tensor_copy(ou[:], ops_t[0:64, :])
                den = smp.tile([1, 1024], f32, tag="den")
                nc.vector.tensor_copy(den[:], ops_t[64:65, :])
                rec = smp.tile([1, 1024], f32, tag="rq")
                nc.vector.reciprocal_approx_fast(rec[:], den[:])
                return (ou, rec)

            def attn_out_b1(b, qb, st):
                """Normalize (non-PE); emitted at the start of the NEXT
                q-block's attention so the rec/broadcast chain runs in its
                shadow."""
                ou, rec = st
                rb = smp.tile([64, 1024], f32, tag="rb")
                nc.gpsimd.partition_broadcast(rb[:], rec[:])
                oT = osbp.tile([128, QB], bf16, tag="o")
                nc.vector.tensor_tensor(oT[0:64, :], ou[:, 0:512],
                                        rb[:, 0:512], op=OP.mult)
                o1 = osbp.tile([64, QB], bf16, tag="o1")
                nc.vector.tensor_tensor(o1[:], ou[:, 512:1024],
                                        rb[:, 512:1024], op=OP.mult)
                nc.sync.dma_start(oT[64:128, :], o1[:])
                if debug and b == 0 and qb == 0:
                    nc.sync.dma_start(dbg_oT[:], oT[:])
                return oT

            def attn_out_b2_gen(b, qb, oT):
                """Out-projection steps, interleaved into the next q-block's
                PE-idle slack via attn_core(outg=...)."""
                for ts in range(4):
                    r0 = b * N + qb * QB + ts * 128
                    ob = outsbp.tile([128, 1024], f32, tag="ob")
                    for eh in range(2):
                        po = psP.tile([128, 512], f32, tag="psp")
                        nc.tensor.matmul(po[:],
                                         oT[:, ts * 128:(ts + 1) * 128],
                                         wo_sb[:, eh * 512:(eh + 1) * 512],
                                         start=True, stop=True)
                        nc.vector.tensor_copy(ob[:, eh * 512:(eh + 1) * 512], po[:])
                        yield
                    nc.sync.dma_start(out_d[r0:r0 + 128, :], ob[:])

            def batch_tiles(b):
                qT = batchp.tile([128, N], bf16, tag="qT")
                kT = batchp.tile([128, N], bf16, tag="kT")
                v_sb = batchp.tile([128, NKT * 65], bf16, tag="v")
                if b < 2:
                    # the ones column survives buffer reuse (v writes only
                    # touch cols 0:64 per k-tile); re-memsetting on reuse
                    # would race the pending tail's reads of the old batch
                    ones = v_sb[:].rearrange("p (kt c) -> p kt c", c=65)[:, :, 64:65]
                    nc.vector.memset(ones, 1.0)
                return (qT, kT, v_sb)

            # software-pipelined emission: the PE-idle window during each
            # q-block's ACT-bound score/exp stream is filled with the next
            # batch's projection chunk.
            tiles = batch_tiles(0)
            xT = xT0
            for c in range(NCH):
                for _ in proj_gen(0, c, tiles, xT):
                    pass
            prev_st = None  # (b, qb, (ou, rec)) ready for normalize+outproj
            pending = None  # (b, qb, tiles, ops_t, es, cell) tail not emitted
            for b in range(B):
                nxt = batch_tiles(b + 1) if b + 1 < B else None
                if nxt is not None:
                    xTn = xtp.tile([128, 8 * N], bf16, tag="xT")
                    load_xt(b + 1, xTn)
                for i in range(NQB):
                    g = proj_gen(b + 1, i, nxt, xTn) if nxt is not None else None
                    outg = None
                    if prev_st is not None:
                        pb, pq, pst = prev_st
                        oT = attn_out_b1(pb, pq, pst)
                        outg = attn_out_b2_gen(pb, pq, oT)
                    pend = None
                    if pending is not None:
                        tb, tq, ttiles, tops, tes, tcell = pending
                        pend = tail_gen(ttiles, tops, tes, tcell)
                    o, es_list = attn_core(b, i, tiles, filler=g,
                                           outg=outg, pend=pend)
                    if pend is not None:
                        for _ in pend:
                            pass
                        prev_st = (tb, tq, tcell[0])
                    if g is not None:
                        for _ in g:
                            pass
                    if outg is not None:
                        for _ in outg:
                            pass
                    pending = (b, i, tiles, o, es_list, [])
                if debug and b == 0:
                    qTd, kTd, vd = tiles
                    nc.sync.dma_start(dbg_qT[:], qTd[:])
                    nc.sync.dma_start(dbg_kT[:], kTd[:])
                    nc.sync.dma_start(dbg_v[:], vd[:])
                if nxt is not None:
                    tiles = nxt
                    xT = xTn
            # drain: last q-block's tail, then the two remaining out stages
            tb, tq, ttiles, tops, tes, tcell = pending
            for _ in tail_gen(ttiles, tops, tes, tcell):
                pass
            if prev_st is not None:
                pb, pq, pst = prev_st
                oT = attn_out_b1(pb, pq, pst)
                for _ in attn_out_b2_gen(pb, pq, oT):
                    pass
            oT = attn_out_b1(tb, tq, tcell[0])
            for _ in attn_out_b2_gen(tb, tq, oT):
                pass

    nc.compile()
    _BUILT[("nc", debug)] = nc
    return nc


def _make_in_maps(x, Wq, Wk, Wv, Wo):
    import ml_dtypes
    bf16 = ml_dtypes.bfloat16
    cos_t, sin_t = _rope_tables()
    cos_t = cos_t.astype(bf16)
    sin_t = sin_t.astype(bf16)
    xt = np.ascontiguousarray(
        np.asarray(x, np.float32).reshape(T, DIM).T).astype(bf16)

    def stack8(w):  # [1024, 128] -> [128, 8*128] with w[et*128+p, j] at [p, et*128+j]
        return np.ascontiguousarray(
            w.reshape(8, 128, 128).transpose(1, 0, 2).reshape(128, DIM))

    in_maps = []
    for d in range(N_CORES):
        wq_d = stack8(Wq[:, d * 128:(d + 1) * 128]).astype(bf16)
        wk_d = Wk[:, d * 64:(d + 1) * 64]
        wv_d = Wv[:, d * 64:(d + 1) * 64]
        wkv_d = stack8(np.concatenate([wk_d, wv_d], axis=1)).astype(bf16)
        wo_d = np.ascontiguousarray(Wo[d * 128:(d + 1) * 128, :]).astype(bf16)
        in_maps.append({
            "xt": xt, "wq": wq_d, "wkv": wkv_d, "wo": wo_d,
            "cos_t": cos_t, "sin_t": sin_t,
        })
    return in_maps


def _run(in_maps, trace=False, trace_kwargs=None, debug=False):
    _ensure_path()
    from concourse.bass_utils import run_bass_kernel_spmd
    nc = _build(debug=debug)
    return run_bass_kernel_spmd(nc, in_maps, list(range(N_CORES)), trace=trace,
                                **(trace_kwargs or {}))


def kernel(x, Wq, Wk, Wv, Wo, bo):
    x = np.asarray(x, dtype=np.float32)
    in_maps = _make_in_maps(x, np.asarray(Wq, np.float32), np.asarray(Wk, np.float32),
                            np.asarray(Wv, np.float32), np.asarray(Wo, np.float32))
    res = _run(in_maps)
    acc = np.zeros((T, DIM), dtype=np.float32)
    for d in range(N_CORES):
        acc += np.asarray(res.results[d]["out"]).astype(np.float32)
    acc += np.asarray(bo, np.float32)[None, :]
    return acc.reshape(B, N, DIM)
